# revision 15
# baseline (speedup 1.0000x reference)
"""Bass/Trainium2 kernel for nn_BaselineLSTM (B=2048, T=512, H=128, twin=256).

Strategy:
  - Data-parallel: batch 2048 -> 8 cores x 256; each core runs 2 interleaved
    chunks of 128 batch (pipelining hides per-step cross-engine latency).
  - State kept transposed: hT/cT = [H=128 partitions, batch free]. Gate
    matmuls are out[gate_rows, batch] = W_slice.T.T @ hT -> no per-step
    transpose anywhere.
  - Gates land in one PSUM bank per chunk-step ordered [i|f|o|g] so a single
    merged Sigmoid ACT covers i,f,o; Tanh covers g.
  - Phase P (teacher forcing): input + bias enter via a K=2 accumulating
    matmul against packed rows [y_t; 1].
  - Phase H (autoregressive): x_t = W_out h + b_out is folded into the
    recurrence:  g = (W_hh + W_ih W_out) h + (b + W_ih b_out). No feedback
    data path; bias enters via a K=1 matmul against a ones row.
  - Predictions p_t = W_out h_t accumulate into a PSUM bank via M=1 matmuls
    (one row per step), flushed to DRAM every EPOCH=128 steps. b_out is added
    on the host.
"""

import functools

import ml_dtypes
import numpy as np

import concourse.bacc as bacc
import concourse.tile as tile
from concourse import mybir
from concourse.bass_utils import run_bass_kernel_spmd

F32 = mybir.dt.float32
BF16 = mybir.dt.bfloat16
AF = mybir.ActivationFunctionType

H = 128          # hidden
NCORES = 8
BS = 256         # batch per core
BC = 128         # batch per chunk
NCHUNK = 2

# pytorch gate order (i, f, g, o) -> kernel order (i, f, o, g)
_PERM = np.concatenate([np.arange(0, 128), np.arange(128, 256),
                        np.arange(384, 512), np.arange(256, 384)])


def _build_body(tc, d, NP, NH, EPOCH):
    nc = tc.nc
    NT = NP + NH
    NEP = (NT + EPOCH - 1) // EPOCH

    import contextlib
    with contextlib.ExitStack() as ctx:
        consts = ctx.enter_context(tc.tile_pool(name="consts", bufs=1))
        state = ctx.enter_context(tc.tile_pool(name="state", bufs=1))
        work = ctx.enter_context(tc.tile_pool(name="work", bufs=3))
        gpool = ctx.enter_context(tc.tile_pool(name="gates", bufs=2, space="PSUM"))
        ppool = ctx.enter_context(tc.tile_pool(name="ppsum", bufs=2, space="PSUM"))

        # ---- constants to SBUF
        whhT_p = consts.tile([H, 4 * H], BF16, tag="whhT_p")
        whhT_h = consts.tile([H, 4 * H], BF16, tag="whhT_h")
        lp = consts.tile([2, 4 * H], BF16, tag="lp")
        lh = consts.tile([1, 4 * H], BF16, tag="lh")
        woutZ = consts.tile([H, 2 * H], BF16, tag="woutZ")
        xq = consts.tile([2, NP * BS], BF16, tag="xq")
        ones = consts.tile([1, BS], BF16, tag="ones")
        nc.vector.memset(ones, 1.0)
        nc.sync.dma_start(out=whhT_p, in_=d["whhT_p"])
        nc.sync.dma_start(out=whhT_h, in_=d["whhT_h"])
        nc.sync.dma_start(out=lp, in_=d["lp"])
        nc.sync.dma_start(out=lh, in_=d["lh"])
        nc.sync.dma_start(out=woutZ, in_=d["woutZ"])
        nc.sync.dma_start(out=xq, in_=d["xq"])

        # ---- state
        hT = []
        cT = []
        for ch in range(NCHUNK):
            h = state.tile([H, BC], BF16, tag=f"hT{ch}")
            c = state.tile([H, BC], F32, tag=f"cT{ch}")
            nc.vector.memset(h, 0.0)
            nc.vector.memset(c, 0.0)
            hT.append(h)
            cT.append(c)

        pps = None

        def step(s, ch):
            phase_p = s < NP
            gates = gpool.tile([H, 4 * H], F32, tag=f"g{ch}")
            whh = whhT_p if phase_p else whhT_h
            for k in range(4):
                go = gates[:, k * H:(k + 1) * H]
                nc.tensor.matmul(go, whh[:, k * H:(k + 1) * H], hT[ch],
                                 start=True, stop=False)
                if phase_p:
                    rhs = xq[0:2, s * BS + ch * BC: s * BS + ch * BC + BC]
                    lhs2 = lp[0:2, k * H:(k + 1) * H]
                else:
                    rhs = ones[0:1, ch * BC: ch * BC + BC]
                    lhs2 = lh[0:1, k * H:(k + 1) * H]
                nc.tensor.matmul(go, lhs2, rhs, start=False, stop=True)

            sig = work.tile([H, 3 * H], BF16, tag=f"sig{ch}")
            nc.scalar.activation(sig, gates[:, 0:3 * H], AF.Sigmoid)
            tg = work.tile([H, BC], BF16, tag=f"tg{ch}")
            nc.scalar.activation(tg, gates[:, 3 * H:4 * H], AF.Tanh)

            t2 = work.tile([H, BC], F32, tag=f"t2{ch}")
            nc.vector.tensor_mul(t2, sig[:, H:2 * H], cT[ch])
            t1 = work.tile([H, BC], BF16, tag=f"t1{ch}")
            nc.gpsimd.tensor_mul(t1, sig[:, 0:H], tg)
            nc.vector.tensor_add(cT[ch], t2, t1)
            tcn = work.tile([H, BC], BF16, tag=f"tcn{ch}")
            nc.scalar.activation(tcn, cT[ch], AF.Tanh)
            nc.vector.tensor_mul(hT[ch], sig[:, 2 * H:3 * H], tcn)

            # p_s = W_out @ h lands in PSUM row r via a shifted zero-padded
            # stationary matrix (rows != r accumulate exact zeros). One PSUM
            # bank per chunk: start=True resets the whole bank.
            r = s % EPOCH
            nc.tensor.matmul(pps[ch][:, :],
                             woutZ[:, H - r: 2 * H - r], hT[ch],
                             start=(r == 0), stop=(r == EPOCH - 1 or s == NT - 1),
                             skip_group_check=True)

        for s in range(NT):
            if s % EPOCH == 0:
                pps = [ppool.tile([H, BC], F32, tag=f"pps{ch}",
                                  name=f"pps{ch}_{s}")
                       for ch in range(NCHUNK)]
            for ch in range(NCHUNK):
                step(s, ch)
            if s % EPOCH == EPOCH - 1 or s == NT - 1:
                e = s // EPOCH
                pc = work.tile([EPOCH, BS], F32, tag="pc")
                for ch in range(NCHUNK):
                    nc.vector.tensor_copy(pc[:, ch * BC:(ch + 1) * BC],
                                          pps[ch][0:EPOCH, :])
                nc.sync.dma_start(out=d["preds"][e], in_=pc)


@functools.lru_cache(maxsize=2)
def _program(NP, NH, EPOCH):
    nc = bacc.Bacc("TRN2", target_bir_lowering=False, debug=False,
                   num_devices=NCORES)
    NT = NP + NH
    NEP = (NT + EPOCH - 1) // EPOCH
    d = {
        "whhT_p": nc.dram_tensor("whhT_p", [H, 4 * H], BF16,
                                 kind="ExternalInput").ap(),
        "whhT_h": nc.dram_tensor("whhT_h", [H, 4 * H], BF16,
                                 kind="ExternalInput").ap(),
        "lp": nc.dram_tensor("lp", [2, 4 * H], BF16, kind="ExternalInput").ap(),
        "lh": nc.dram_tensor("lh", [1, 4 * H], BF16, kind="ExternalInput").ap(),
        "woutZ": nc.dram_tensor("woutZ", [H, 2 * H], BF16,
                                kind="ExternalInput").ap(),
        "xq": nc.dram_tensor("xq", [2, NP * BS], BF16,
                             kind="ExternalInput").ap(),
        "preds": nc.dram_tensor("preds", [NEP, EPOCH, BS], F32,
                                kind="ExternalOutput").ap(),
    }
    with tile.TileContext(nc) as tc:
        _build_body(tc, d, NP, NH, EPOCH)
    nc.compile()
    return nc


def _host_prep(y_flow, W_ih, W_hh, b_ih, b_hh, W_out, b_out, NP):
    """Build per-core input maps. y_flow: (B, T, 1) f32."""
    bf = ml_dtypes.bfloat16
    W_ih = np.asarray(W_ih, np.float32)
    W_hh = np.asarray(W_hh, np.float32)
    W_out = np.asarray(W_out, np.float32)
    bias = np.asarray(b_ih, np.float32) + np.asarray(b_hh, np.float32)
    b_out = np.asarray(b_out, np.float32)

    W_hh_H = W_hh + W_ih @ W_out          # [4H, H]
    bias_H = bias + W_ih[:, 0] * b_out[0]

    whhT_p = np.ascontiguousarray(W_hh[_PERM].T).astype(bf)      # [H, 4H]
    whhT_h = np.ascontiguousarray(W_hh_H[_PERM].T).astype(bf)
    lp = np.stack([W_ih[_PERM, 0], bias[_PERM]]).astype(bf)       # [2, 4H]
    lh = bias_H[_PERM][None, :].astype(bf)                        # [1, 4H]
    woutZ = np.zeros((H, 2 * H), np.float32)                      # [H, 256]
    woutZ[:, H] = W_out[0]
    woutZ = woutZ.astype(bf)

    y = np.asarray(y_flow, np.float32)[:, :, 0]                   # [B, T]
    B = y.shape[0]
    in_maps = []
    for core in range(NCORES):
        yc = y[core * BS:(core + 1) * BS]                         # [BS, T]
        xq = np.ones((2, NP * BS), np.float32)
        xq[0] = yc[:, :NP].T.reshape(-1)
        in_maps.append({
            "whhT_p": whhT_p, "whhT_h": whhT_h, "lp": lp, "lh": lh,
            "woutZ": woutZ, "xq": xq.astype(bf),
        })
    return in_maps


def kernel(y_flow, x_dyn, W_ih, W_hh, b_ih, b_hh, W_out, b_out, twin_idx,
           _trace=False):
    twin = int(twin_idx)
    assert twin == 256, f"kernel hardcodes twin_idx=256, got {twin}"
    B, T, _ = y_flow.shape
    assert (B, T) == (2048, 512)
    NP, NH, EPOCH = twin - 1, T - twin, 128
    NT = NP + NH

    nc = _program(NP, NH, EPOCH)
    in_maps = _host_prep(y_flow, W_ih, W_hh, b_ih, b_hh, W_out, b_out, NP)
    res = run_bass_kernel_spmd(nc, in_maps, core_ids=list(range(NCORES)),
                               trace=_trace)

    b_out = np.asarray(b_out, np.float32)
    out = np.empty((B, NT, 1), np.float32)
    for core in range(NCORES):
        p = np.asarray(res.results[core]["preds"], np.float32)
        p = p.reshape(-1, BS)[:NT]          # [NT, BS]
        out[core * BS:(core + 1) * BS, :, 0] = p.T + b_out[0]
    if _trace:
        kernel._last_results = res
    return out


# revision 21
# speedup vs baseline: 1.0151x; 1.0151x over previous
"""Bass/Trainium2 kernel for nn_BaselineLSTM (B=2048, T=512, H=128, twin=256).

Strategy:
  - Data-parallel: batch 2048 -> 8 cores x 256; each core runs 2 interleaved
    chunks of 128 batch (pipelining hides per-step cross-engine latency).
  - State kept transposed: hT/cT = [H=128 partitions, batch free]. Gate
    matmuls are out[gate_rows, batch] = W_slice.T.T @ hT -> no per-step
    transpose anywhere.
  - Gates land in one PSUM bank per chunk-step ordered [i|f|o|g] so a single
    merged Sigmoid ACT covers i,f,o; Tanh covers g.
  - Phase P (teacher forcing): input + bias enter via a K=2 accumulating
    matmul against packed rows [y_t; 1].
  - Phase H (autoregressive): x_t = W_out h + b_out is folded into the
    recurrence:  g = (W_hh + W_ih W_out) h + (b + W_ih b_out). No feedback
    data path; bias enters via a K=1 matmul against a ones row.
  - Predictions p_t = W_out h_t accumulate into a PSUM bank via M=1 matmuls
    (one row per step), flushed to DRAM every EPOCH=128 steps. b_out is added
    on the host.
"""

import functools

import ml_dtypes
import numpy as np

import concourse.bacc as bacc
import concourse.tile as tile
from concourse import mybir
from concourse.bass_utils import run_bass_kernel_spmd

F32 = mybir.dt.float32
BF16 = mybir.dt.bfloat16
AF = mybir.ActivationFunctionType

H = 128          # hidden
NCORES = 8
BS = 256         # batch per core
BC = 128         # batch per chunk
NCHUNK = 2

# pytorch gate order (i, f, g, o) -> kernel order (i, f, o, g)
_PERM = np.concatenate([np.arange(0, 128), np.arange(128, 256),
                        np.arange(384, 512), np.arange(256, 384)])


def _build_body(tc, d, NP, NH, EPOCH):
    nc = tc.nc
    NT = NP + NH

    import contextlib
    with contextlib.ExitStack() as ctx:
        consts = ctx.enter_context(tc.tile_pool(name="consts", bufs=1))
        state = ctx.enter_context(tc.tile_pool(name="state", bufs=1))
        work = ctx.enter_context(tc.tile_pool(name="work", bufs=3))
        gpool = ctx.enter_context(tc.tile_pool(name="gates", bufs=3, space="PSUM"))
        ppool = ctx.enter_context(tc.tile_pool(name="ppsum", bufs=1, space="PSUM"))

        # ---- constants to SBUF
        whhT_p = consts.tile([H, 4 * H], BF16, tag="whhT_p")
        whhT_h = consts.tile([H, 4 * H], BF16, tag="whhT_h")
        lp = consts.tile([2, 4 * H], BF16, tag="lp")
        lh = consts.tile([1, 4 * H], BF16, tag="lh")
        woutZ = consts.tile([H, 2 * H], BF16, tag="woutZ")
        xq = consts.tile([2, NP * BS], BF16, tag="xq")
        ones = consts.tile([1, BS], BF16, tag="ones")
        nc.vector.memset(ones, 1.0)
        nc.sync.dma_start(out=whhT_p, in_=d["whhT_p"])
        nc.sync.dma_start(out=whhT_h, in_=d["whhT_h"])
        nc.sync.dma_start(out=lp, in_=d["lp"])
        nc.sync.dma_start(out=lh, in_=d["lh"])
        nc.sync.dma_start(out=woutZ, in_=d["woutZ"])
        nc.sync.dma_start(out=xq, in_=d["xq"])

        # ---- state: h kept in a 4-slot ring (slot s%4) so predictions can
        # be batched 4 steps per matmul against consecutive slots.
        hist = []
        cT = []
        for ch in range(NCHUNK):
            hh = state.tile([H, 4 * BC], BF16, tag=f"hist{ch}")
            c = state.tile([H, BC], F32, tag=f"cT{ch}")
            nc.vector.memset(hh, 0.0)
            nc.vector.memset(c, 0.0)
            hist.append(hh)
            cT.append(c)

        pps = [None, None]
        sigs = [None, None]
        tgs = [None, None]
        gates_l = [None, None]

        def front(s, ch):
            """Gate matmuls + sigmoid/tanh activations for step s."""
            phase_p = s < NP
            gates = gpool.tile([H, 4 * H], F32, tag=f"g{ch}",
                               name=f"g{ch}_{s}")
            gates_l[ch] = gates
            whh = whhT_p if phase_p else whhT_h
            hprev = hist[ch][:, ((s - 1) % 4) * BC: ((s - 1) % 4 + 1) * BC]
            for k in range(4):
                go = gates[:, k * H:(k + 1) * H]
                nc.tensor.matmul(go, whh[:, k * H:(k + 1) * H], hprev,
                                 start=True, stop=False)
                if phase_p:
                    rhs = xq[0:2, s * BS + ch * BC: s * BS + ch * BC + BC]
                    lhs2 = lp[0:2, k * H:(k + 1) * H]
                else:
                    rhs = ones[0:1, ch * BC: ch * BC + BC]
                    lhs2 = lh[0:1, k * H:(k + 1) * H]
                nc.tensor.matmul(go, lhs2, rhs, start=False, stop=True)

            sig = work.tile([H, 3 * H], BF16, tag=f"sig{ch}",
                            name=f"sig{ch}_{s}")
            nc.scalar.activation(sig, gates[:, 0:3 * H], AF.Sigmoid)
            tg = work.tile([H, BC], BF16, tag=f"tg{ch}", name=f"tg{ch}_{s}")
            nc.scalar.activation(tg, gates[:, 3 * H:4 * H], AF.Tanh)
            sigs[ch] = sig
            tgs[ch] = tg

        def back(s, ch):
            """c/h update for step s + batched prediction matmul."""
            sig, tg = sigs[ch], tgs[ch]
            t2 = work.tile([H, BC], F32, tag=f"t2{ch}", name=f"t2{ch}_{s}")
            nc.vector.tensor_mul(t2, sig[:, H:2 * H], cT[ch])
            t1 = work.tile([H, BC], BF16, tag=f"t1{ch}", name=f"t1{ch}_{s}")
            nc.gpsimd.tensor_mul(t1, sig[:, 0:H], tg)
            nc.vector.tensor_add(cT[ch], t2, t1)
            tcn = work.tile([H, BC], BF16, tag=f"tcn{ch}", name=f"tcn{ch}_{s}")
            nc.scalar.activation(tcn, cT[ch], AF.Tanh)
            hslot = hist[ch][:, (s % 4) * BC: (s % 4 + 1) * BC]
            nc.gpsimd.tensor_mul(hslot, sig[:, 2 * H:3 * H], tcn)

            # Predictions: every 4 steps, p for steps 4G..4G+3 = one matmul
            # W_out @ [h_0|h_1|h_2|h_3]; row placement via shifted zero-pad.
            if s % 4 == 3 or s == NT - 1:
                G = s // 4
                r = G % 32
                n = (s % 4 + 1) * BC
                if r == 0:
                    pps[ch] = ppool.tile([H, 4 * BC], F32, tag=f"pps{ch}",
                                         name=f"pps{ch}_{s}")
                nc.tensor.matmul(pps[ch][:, 0:n],
                                 woutZ[:, H - r: 2 * H - r],
                                 hist[ch][:, 0:n],
                                 start=(r == 0), stop=(r == 31 or s == NT - 1),
                                 skip_group_check=True)
                if r == 31 or s == NT - 1:
                    e = G // 32
                    pc = work.tile([32, 4 * BC], F32, tag=f"pc{ch}",
                                   name=f"pc{ch}_{s}")
                    nc.vector.tensor_copy(pc, pps[ch][0:32, :])
                    nc.sync.dma_start(out=d["preds"][e, ch], in_=pc)

        # Software pipeline: chunk 1 runs half a step behind chunk 0 so
        # engines ping-pong between the two independent recurrences.
        for s in range(NT):
            front(s, 0)
            if s > 0:
                back(s - 1, 1)
            front(s, 1)
            back(s, 0)
        back(NT - 1, 1)


@functools.lru_cache(maxsize=2)
def _program(NP, NH, EPOCH):
    nc = bacc.Bacc("TRN2", target_bir_lowering=False, debug=False,
                   num_devices=NCORES)
    NT = NP + NH
    NEP = (NT + 127) // 128
    d = {
        "whhT_p": nc.dram_tensor("whhT_p", [H, 4 * H], BF16,
                                 kind="ExternalInput").ap(),
        "whhT_h": nc.dram_tensor("whhT_h", [H, 4 * H], BF16,
                                 kind="ExternalInput").ap(),
        "lp": nc.dram_tensor("lp", [2, 4 * H], BF16, kind="ExternalInput").ap(),
        "lh": nc.dram_tensor("lh", [1, 4 * H], BF16, kind="ExternalInput").ap(),
        "woutZ": nc.dram_tensor("woutZ", [H, 2 * H], BF16,
                                kind="ExternalInput").ap(),
        "xq": nc.dram_tensor("xq", [2, NP * BS], BF16,
                             kind="ExternalInput").ap(),
        "preds": nc.dram_tensor("preds", [NEP, NCHUNK, 32, 4 * BC], F32,
                                kind="ExternalOutput").ap(),
    }
    with tile.TileContext(nc) as tc:
        _build_body(tc, d, NP, NH, EPOCH)
    nc.compile()
    return nc


def _host_prep(y_flow, W_ih, W_hh, b_ih, b_hh, W_out, b_out, NP):
    """Build per-core input maps. y_flow: (B, T, 1) f32."""
    bf = ml_dtypes.bfloat16
    W_ih = np.asarray(W_ih, np.float32)
    W_hh = np.asarray(W_hh, np.float32)
    W_out = np.asarray(W_out, np.float32)
    bias = np.asarray(b_ih, np.float32) + np.asarray(b_hh, np.float32)
    b_out = np.asarray(b_out, np.float32)

    W_hh_H = W_hh + W_ih @ W_out          # [4H, H]
    bias_H = bias + W_ih[:, 0] * b_out[0]

    whhT_p = np.ascontiguousarray(W_hh[_PERM].T).astype(bf)      # [H, 4H]
    whhT_h = np.ascontiguousarray(W_hh_H[_PERM].T).astype(bf)
    lp = np.stack([W_ih[_PERM, 0], bias[_PERM]]).astype(bf)       # [2, 4H]
    lh = bias_H[_PERM][None, :].astype(bf)                        # [1, 4H]
    woutZ = np.zeros((H, 2 * H), np.float32)                      # [H, 256]
    woutZ[:, H] = W_out[0]
    woutZ = woutZ.astype(bf)

    y = np.asarray(y_flow, np.float32)[:, :, 0]                   # [B, T]
    B = y.shape[0]
    in_maps = []
    for core in range(NCORES):
        yc = y[core * BS:(core + 1) * BS]                         # [BS, T]
        xq = np.ones((2, NP * BS), np.float32)
        xq[0] = yc[:, :NP].T.reshape(-1)
        in_maps.append({
            "whhT_p": whhT_p, "whhT_h": whhT_h, "lp": lp, "lh": lh,
            "woutZ": woutZ, "xq": xq.astype(bf),
        })
    return in_maps


def kernel(y_flow, x_dyn, W_ih, W_hh, b_ih, b_hh, W_out, b_out, twin_idx,
           _trace=False):
    twin = int(twin_idx)
    assert twin == 256, f"kernel hardcodes twin_idx=256, got {twin}"
    B, T, _ = y_flow.shape
    assert (B, T) == (2048, 512)
    NP, NH, EPOCH = twin - 1, T - twin, 128
    NT = NP + NH

    nc = _program(NP, NH, EPOCH)
    in_maps = _host_prep(y_flow, W_ih, W_hh, b_ih, b_hh, W_out, b_out, NP)
    res = run_bass_kernel_spmd(nc, in_maps, core_ids=list(range(NCORES)),
                               trace=_trace)

    b_out = np.asarray(b_out, np.float32)
    out = np.empty((B, NT, 1), np.float32)
    for core in range(NCORES):
        p = np.asarray(res.results[core]["preds"], np.float32)
        nep = p.shape[0]
        a = p.reshape(nep, NCHUNK, 32, 4, BC)      # [e, ch, r, j, b]
        for ch in range(NCHUNK):
            blk = a[:, ch].transpose(3, 0, 1, 2).reshape(BC, -1)[:, :NT]
            out[core * BS + ch * BC: core * BS + (ch + 1) * BC, :, 0] = \
                blk + b_out[0]
    if _trace:
        kernel._last_results = res
    return out


# revision 24
# speedup vs baseline: 1.1753x; 1.1578x over previous
"""Bass/Trainium2 kernel for nn_BaselineLSTM (B=2048, T=512, H=128, twin=256).

Strategy:
  - Data-parallel: batch 2048 -> 8 cores x 256; each core runs 2 interleaved
    chunks of 128 batch (pipelining hides per-step cross-engine latency).
  - State kept transposed: hT/cT = [H=128 partitions, batch free]. Gate
    matmuls are out[gate_rows, batch] = W_slice.T.T @ hT -> no per-step
    transpose anywhere.
  - Gates land in one PSUM bank per chunk-step ordered [i|f|o|g] so a single
    merged Sigmoid ACT covers i,f,o; Tanh covers g.
  - Phase P (teacher forcing): input + bias enter via a K=2 accumulating
    matmul against packed rows [y_t; 1].
  - Phase H (autoregressive): x_t = W_out h + b_out is folded into the
    recurrence:  g = (W_hh + W_ih W_out) h + (b + W_ih b_out). No feedback
    data path; bias enters via a K=1 matmul against a ones row.
  - Predictions p_t = W_out h_t accumulate into a PSUM bank via M=1 matmuls
    (one row per step), flushed to DRAM every EPOCH=128 steps. b_out is added
    on the host.
"""

import functools

import ml_dtypes
import numpy as np

import concourse.bacc as bacc
import concourse.tile as tile
from concourse import mybir
from concourse.bass_utils import run_bass_kernel_spmd

F32 = mybir.dt.float32
BF16 = mybir.dt.bfloat16
AF = mybir.ActivationFunctionType

H = 128          # hidden
NCORES = 8
BS = 256         # batch per core
BC = 128         # batch per chunk
NCHUNK = 2

# pytorch gate order (i, f, g, o) -> kernel order (i, f, o, g)
_PERM = np.concatenate([np.arange(0, 128), np.arange(128, 256),
                        np.arange(384, 512), np.arange(256, 384)])


def _build_body(tc, d, NP, NH, EPOCH):
    nc = tc.nc
    NT = NP + NH

    import contextlib
    with contextlib.ExitStack() as ctx:
        consts = ctx.enter_context(tc.tile_pool(name="consts", bufs=1))
        state = ctx.enter_context(tc.tile_pool(name="state", bufs=1))
        work = ctx.enter_context(tc.tile_pool(name="work", bufs=3))
        gpool = ctx.enter_context(tc.tile_pool(name="gates", bufs=3, space="PSUM"))
        ppool = ctx.enter_context(tc.tile_pool(name="ppsum", bufs=1, space="PSUM"))

        # ---- constants to SBUF
        whhT_p = consts.tile([H, 4 * H], BF16, tag="whhT_p")
        whhT_h = consts.tile([H, 4 * H], BF16, tag="whhT_h")
        lp = consts.tile([2, 4 * H], BF16, tag="lp")
        lh = consts.tile([1, 4 * H], BF16, tag="lh")
        woutZ = consts.tile([H, 2 * H], BF16, tag="woutZ")
        xq = consts.tile([2, NP * BS], BF16, tag="xq")
        ones = consts.tile([1, BS], BF16, tag="ones")
        nc.vector.memset(ones, 1.0)
        nc.sync.dma_start(out=whhT_p, in_=d["whhT_p"])
        nc.sync.dma_start(out=whhT_h, in_=d["whhT_h"])
        nc.sync.dma_start(out=lp, in_=d["lp"])
        nc.sync.dma_start(out=lh, in_=d["lh"])
        nc.sync.dma_start(out=woutZ, in_=d["woutZ"])
        nc.sync.dma_start(out=xq, in_=d["xq"])

        # ---- state: h kept in a 4-slot ring (slot s%4) so predictions can
        # be batched 4 steps per matmul against consecutive slots.
        hist = []
        cT = []
        for ch in range(NCHUNK):
            hh = state.tile([H, 4 * BC], BF16, tag=f"hist{ch}")
            c = state.tile([H, BC], BF16, tag=f"cT{ch}")
            nc.vector.memset(hh, 0.0)
            nc.vector.memset(c, 0.0)
            hist.append(hh)
            cT.append(c)

        pps = [None, None]
        sigs = [None, None]
        tgs = [None, None]
        gates_l = [None, None]

        def front(s, ch):
            """Gate matmuls + sigmoid/tanh activations for step s."""
            phase_p = s < NP
            gates = gpool.tile([H, 4 * H], F32, tag=f"g{ch}",
                               name=f"g{ch}_{s}")
            gates_l[ch] = gates
            whh = whhT_p if phase_p else whhT_h
            hprev = hist[ch][:, ((s - 1) % 4) * BC: ((s - 1) % 4 + 1) * BC]

            def gate_mm(k):
                go = gates[:, k * H:(k + 1) * H]
                nc.tensor.matmul(go, whh[:, k * H:(k + 1) * H], hprev,
                                 start=True, stop=False)
                if phase_p:
                    rhs = xq[0:2, s * BS + ch * BC: s * BS + ch * BC + BC]
                    lhs2 = lp[0:2, k * H:(k + 1) * H]
                else:
                    rhs = ones[0:1, ch * BC: ch * BC + BC]
                    lhs2 = lh[0:1, k * H:(k + 1) * H]
                nc.tensor.matmul(go, lhs2, rhs, start=False, stop=True)

            # g-gate first so tanh(g) can run on ACT while i/f/o matmuls
            # are still streaming; sigmoid follows.
            gate_mm(3)
            tg = work.tile([H, BC], BF16, tag=f"tg{ch}", name=f"tg{ch}_{s}")
            nc.scalar.activation(tg, gates[:, 3 * H:4 * H], AF.Tanh)
            for k in (0, 1, 2):
                gate_mm(k)
            sig = work.tile([H, 3 * H], BF16, tag=f"sig{ch}",
                            name=f"sig{ch}_{s}")
            nc.scalar.activation(sig, gates[:, 0:3 * H], AF.Sigmoid)
            sigs[ch] = sig
            tgs[ch] = tg

        def back(s, ch):
            """c/h update for step s + batched prediction matmul."""
            sig, tg = sigs[ch], tgs[ch]
            t2 = work.tile([H, BC], BF16, tag=f"t2{ch}", name=f"t2{ch}_{s}")
            nc.vector.tensor_mul(t2, sig[:, H:2 * H], cT[ch])
            t1 = work.tile([H, BC], BF16, tag=f"t1{ch}", name=f"t1{ch}_{s}")
            nc.gpsimd.tensor_mul(t1, sig[:, 0:H], tg)
            nc.vector.tensor_add(cT[ch], t2, t1)
            tcn = work.tile([H, BC], BF16, tag=f"tcn{ch}", name=f"tcn{ch}_{s}")
            nc.scalar.activation(tcn, cT[ch], AF.Tanh)
            hslot = hist[ch][:, (s % 4) * BC: (s % 4 + 1) * BC]
            nc.vector.tensor_mul(hslot, sig[:, 2 * H:3 * H], tcn)

            # Predictions: every 4 steps, p for steps 4G..4G+3 = one matmul
            # W_out @ [h_0|h_1|h_2|h_3]; row placement via shifted zero-pad.
            if s % 4 == 3 or s == NT - 1:
                G = s // 4
                r = G % 32
                n = (s % 4 + 1) * BC
                if r == 0:
                    pps[ch] = ppool.tile([H, 4 * BC], F32, tag=f"pps{ch}",
                                         name=f"pps{ch}_{s}")
                nc.tensor.matmul(pps[ch][:, 0:n],
                                 woutZ[:, H - r: 2 * H - r],
                                 hist[ch][:, 0:n],
                                 start=(r == 0), stop=(r == 31 or s == NT - 1),
                                 skip_group_check=True)
                if r == 31 or s == NT - 1:
                    e = G // 32
                    pc = work.tile([32, 4 * BC], F32, tag=f"pc{ch}",
                                   name=f"pc{ch}_{s}")
                    nc.vector.tensor_copy(pc, pps[ch][0:32, :])
                    nc.sync.dma_start(out=d["preds"][e, ch], in_=pc)

        # Software pipeline: chunk 1 runs half a step behind chunk 0 so
        # engines ping-pong between the two independent recurrences.
        for s in range(NT):
            front(s, 0)
            if s > 0:
                back(s - 1, 1)
            front(s, 1)
            back(s, 0)
        back(NT - 1, 1)


@functools.lru_cache(maxsize=2)
def _program(NP, NH, EPOCH):
    nc = bacc.Bacc("TRN2", target_bir_lowering=False, debug=False,
                   num_devices=NCORES)
    NT = NP + NH
    NEP = (NT + 127) // 128
    d = {
        "whhT_p": nc.dram_tensor("whhT_p", [H, 4 * H], BF16,
                                 kind="ExternalInput").ap(),
        "whhT_h": nc.dram_tensor("whhT_h", [H, 4 * H], BF16,
                                 kind="ExternalInput").ap(),
        "lp": nc.dram_tensor("lp", [2, 4 * H], BF16, kind="ExternalInput").ap(),
        "lh": nc.dram_tensor("lh", [1, 4 * H], BF16, kind="ExternalInput").ap(),
        "woutZ": nc.dram_tensor("woutZ", [H, 2 * H], BF16,
                                kind="ExternalInput").ap(),
        "xq": nc.dram_tensor("xq", [2, NP * BS], BF16,
                             kind="ExternalInput").ap(),
        "preds": nc.dram_tensor("preds", [NEP, NCHUNK, 32, 4 * BC], F32,
                                kind="ExternalOutput").ap(),
    }
    with tile.TileContext(nc) as tc:
        _build_body(tc, d, NP, NH, EPOCH)
    nc.compile()
    return nc


def _host_prep(y_flow, W_ih, W_hh, b_ih, b_hh, W_out, b_out, NP):
    """Build per-core input maps. y_flow: (B, T, 1) f32."""
    bf = ml_dtypes.bfloat16
    W_ih = np.asarray(W_ih, np.float32)
    W_hh = np.asarray(W_hh, np.float32)
    W_out = np.asarray(W_out, np.float32)
    bias = np.asarray(b_ih, np.float32) + np.asarray(b_hh, np.float32)
    b_out = np.asarray(b_out, np.float32)

    W_hh_H = W_hh + W_ih @ W_out          # [4H, H]
    bias_H = bias + W_ih[:, 0] * b_out[0]

    whhT_p = np.ascontiguousarray(W_hh[_PERM].T).astype(bf)      # [H, 4H]
    whhT_h = np.ascontiguousarray(W_hh_H[_PERM].T).astype(bf)
    lp = np.stack([W_ih[_PERM, 0], bias[_PERM]]).astype(bf)       # [2, 4H]
    lh = bias_H[_PERM][None, :].astype(bf)                        # [1, 4H]
    woutZ = np.zeros((H, 2 * H), np.float32)                      # [H, 256]
    woutZ[:, H] = W_out[0]
    woutZ = woutZ.astype(bf)

    y = np.asarray(y_flow, np.float32)[:, :, 0]                   # [B, T]
    B = y.shape[0]
    in_maps = []
    for core in range(NCORES):
        yc = y[core * BS:(core + 1) * BS]                         # [BS, T]
        xq = np.ones((2, NP * BS), np.float32)
        xq[0] = yc[:, :NP].T.reshape(-1)
        in_maps.append({
            "whhT_p": whhT_p, "whhT_h": whhT_h, "lp": lp, "lh": lh,
            "woutZ": woutZ, "xq": xq.astype(bf),
        })
    return in_maps


def kernel(y_flow, x_dyn, W_ih, W_hh, b_ih, b_hh, W_out, b_out, twin_idx,
           _trace=False):
    twin = int(twin_idx)
    assert twin == 256, f"kernel hardcodes twin_idx=256, got {twin}"
    B, T, _ = y_flow.shape
    assert (B, T) == (2048, 512)
    NP, NH, EPOCH = twin - 1, T - twin, 128
    NT = NP + NH

    nc = _program(NP, NH, EPOCH)
    in_maps = _host_prep(y_flow, W_ih, W_hh, b_ih, b_hh, W_out, b_out, NP)
    res = run_bass_kernel_spmd(nc, in_maps, core_ids=list(range(NCORES)),
                               trace=_trace)

    b_out = np.asarray(b_out, np.float32)
    out = np.empty((B, NT, 1), np.float32)
    for core in range(NCORES):
        p = np.asarray(res.results[core]["preds"], np.float32)
        nep = p.shape[0]
        a = p.reshape(nep, NCHUNK, 32, 4, BC)      # [e, ch, r, j, b]
        for ch in range(NCHUNK):
            blk = a[:, ch].transpose(3, 0, 1, 2).reshape(BC, -1)[:, :NT]
            out[core * BS + ch * BC: core * BS + (ch + 1) * BC, :, 0] = \
                blk + b_out[0]
    if _trace:
        kernel._last_results = res
    return out
